# revision 25
# baseline (speedup 1.0000x reference)
"""Masked dot-product attention on 8 Trainium2 NeuronCores (valid-rows-only).

Full inputs: queries/keys/values [16, 2048, 128] f32, valid_lens [16] int.
The reference masks whole query rows q >= valid_len (softmax of a constant
row = uniform weights = mean(V)), so only sum(valid_lens) ~ 47% of rows
need real attention.  Masked rows are filled with mean(V) on the host.

Device-side plan (single SPMD program shared by all 8 cores):
  - Host cuts each batch's valid rows into 512-row and 128-row sections
    and deals them to cores so every core runs exactly NA 512-units and
    NBM 128-units (identical static program; a few padded units).
  - Per-core data is fully pre-staged by the host in fp16: qT [128d, ROWS]
    (sections concatenated column-wise), and PER-UNIT private copies of
    that unit's K^T [128d, 2048k] and [V|1] [128k, 16kt, 129] (duplicated
    when consecutive units share a batch) - the program is fully static,
    no runtime branching or register-offset APs.
  - Per unit: S^T[k,q] = kT . qT on PE into [128, <=3, w] psum chunks;
    exp via one scalar activation per chunk (N<=1536); PV[q,129] =
    E_chunk^T . [V|1] on PE (E stationary, 16 accumulating matmuls per
    128-row qsub into a bank-aligned [128,129] psum); DVE divides by the
    last column; fp16 out, descrambled on the host.
"""

import math
from contextlib import ExitStack

import numpy as np

import concourse.bacc as bacc
import concourse.bass as bass
import concourse.tile as tile
from concourse import mybir
from concourse.bass_utils import run_bass_kernel_spmd

B, Q, K, D = 16, 2048, 2048, 128
NCORES = 8
P = 128
NKT = K // P                 # 16 k-tiles
WBIG = 512                   # big unit width (4 qsubs)
WSM = 128                    # small unit width
SCALE = 1.0 / math.sqrt(D)

F32 = mybir.dt.float32
F16 = mybir.dt.float16


# ----------------------------------------------------------------------------
# host-side planning
# ----------------------------------------------------------------------------

def _sections(vl):
    out = []
    for v in vl:
        a = v // WBIG
        rem = v - a * WBIG
        nb_ = (rem + WSM - 1) // WSM
        if nb_ * WSM >= WBIG:
            a += 1
            nb_ = 0
        out.append((a, nb_))
    return out


def _plan(valid_lens):
    """Returns (na, nbm, cores); cores[c] = unit list (batch, row0, width),
    batch None for padding, big units first."""
    vl = [int(v) for v in valid_lens]
    ab = _sections(vl)
    A = sum(a for a, _ in ab)
    Bs = sum(b for _, b in ab)
    best = None
    for na in range((A + NCORES - 1) // NCORES, -1, -1):
        d = max(0, A - NCORES * na)
        btot = Bs + 4 * d
        nbm = (btot + NCORES - 1) // NCORES
        cost = NCORES * (WBIG * na + WSM * nbm)
        if cost >= WBIG * A + WSM * Bs:
            if best is None or cost < best[0]:
                best = (cost, na, nbm, d)
    _, na, nbm, d = best
    ab = list(ab)
    for b in sorted(range(B), key=lambda x: -ab[x][0]):
        if d == 0:
            break
        a, s = ab[b]
        if a > 0:
            ab[b] = (a - 1, s + 4)
            d -= 1
    big_secs, sm_secs = [], []
    for b in range(B):
        a, s = ab[b]
        for i in range(a):
            big_secs.append((b, i * WBIG))
        for i in range(s):
            sm_secs.append((b, a * WBIG + i * WSM))
    cores = []
    bi = si = 0
    for c in range(NCORES):
        units = []
        for _ in range(na):
            units.append((*big_secs[bi], WBIG) if bi < len(big_secs)
                         else (None, 0, WBIG))
            bi += 1
        for _ in range(nbm):
            units.append((*sm_secs[si], WSM) if si < len(sm_secs)
                         else (None, 0, WSM))
            si += 1
        cores.append(units)
    return na, nbm, cores


def _core_arrays(units, keysT16, vb16, q32, vl):
    rows = sum(w for _, _, w in units)
    nu = len(units)
    qT = np.zeros((P, rows), dtype=np.float16)
    kT = np.zeros((P, nu, K), dtype=np.float16)
    vb = np.zeros((P, nu, NKT, D + 1), dtype=np.float16)
    col = 0
    for i, (b, r0, w) in enumerate(units):
        if b is not None:
            kT[:, i, :] = keysT16[b]
            vb[:, i, :, :] = vb16[b]
            nr = max(0, min(w, vl[b] - r0))
            if nr > 0:
                qT[:, col:col + nr] = q32[b, r0:r0 + nr, :].T
        else:
            vb[:, i, :, D] = 1.0      # keep denominators nonzero on padding
        col += w
    return {"qt": qT, "kt": kT, "vb": vb}


# ----------------------------------------------------------------------------
# bass program (shared across cores; depends only on (na, nbm))
# ----------------------------------------------------------------------------

def _chunks_for(w):
    if w == WBIG:
        return [3, 3, 3, 3, 2, 2]
    return [8, 8]


def _build_program(na, nbm):
    nc = bacc.Bacc(name=f"attn_v_{na}_{nbm}")

    widths = [WBIG] * na + [WSM] * nbm
    nu = len(widths)
    rows = sum(widths)
    totqs = sum(w // P for w in widths)

    qt_d = nc.dram_tensor("qt", [P, rows], F16, kind="ExternalInput")
    kt_d = nc.dram_tensor("kt", [P, nu, K], F16, kind="ExternalInput")
    vb_d = nc.dram_tensor("vb", [P, nu, NKT, D + 1], F16, kind="ExternalInput")
    out_d = nc.dram_tensor("out", [totqs, P, D], F16, kind="ExternalOutput")

    with tile.TileContext(nc) as tc, ExitStack() as ctx:
        sing = ctx.enter_context(tc.tile_pool(name="sing", bufs=1))
        epool = ctx.enter_context(tc.tile_pool(name="epool", bufs=3))
        opool = ctx.enter_context(tc.tile_pool(name="opool", bufs=4))
        rpool = ctx.enter_context(tc.tile_pool(name="rpool", bufs=4))
        ps_s = ctx.enter_context(tc.tile_pool(name="ps_s", bufs=2, space="PSUM"))
        ps_pv = ctx.enter_context(tc.tile_pool(name="ps_pv", bufs=2, space="PSUM"))

        kt_sb = sing.tile([P, nu, K], F16)
        vb_sb = sing.tile([P, nu, NKT, D + 1], F16)
        qt_sb = sing.tile([P, rows], F16)
        # unit 0 head: first kT chunk piece + its qT land first so the first
        # S matmuls start as early as possible; V|1 is only needed at PV time.
        ch0 = _chunks_for(widths[0])[0]
        nc.sync.dma_start(out=kt_sb[:, 0, 0:ch0 * P], in_=kt_d[:, 0, 0:ch0 * P])
        # qT of unit 0 rides the gpsimd queue so it transfers concurrently
        # with unit 0's kT on sync during the one-time DMA warmup
        nc.gpsimd.dma_start(out=qt_sb[:, 0:widths[0]], in_=qt_d[:, 0:widths[0]])
        nc.sync.dma_start(out=kt_sb[:, 0, ch0 * P:K], in_=kt_d[:, 0, ch0 * P:K])
        nc.sync.dma_start(out=vb_sb[:, 0, :, :], in_=vb_d[:, 0, :, :])
        col0 = widths[0]
        for u, w in list(enumerate(widths))[1:]:
            nc.sync.dma_start(out=kt_sb[:, u, :], in_=kt_d[:, u, :])
            nc.sync.dma_start(out=vb_sb[:, u, :, :], in_=vb_d[:, u, :, :])
            nc.sync.dma_start(out=qt_sb[:, col0:col0 + w],
                              in_=qt_d[:, col0:col0 + w])
            col0 += w

        # pending PV drains: {e, u, nqs, oqs, j}
        pending = []

        def emit_pv_step():
            if not pending:
                return
            rec = pending[0]
            j = rec["j"]
            u = rec["u"]
            pv = ps_pv.tile([P, D + 1], F32, tag="pv")
            for kt in range(NKT):
                nc.tensor.matmul(
                    pv,
                    lhsT=rec["e"][:, kt, j * P:(j + 1) * P],
                    rhs=vb_sb[:, u, kt, :],
                    start=(kt == 0),
                    stop=(kt == NKT - 1),
                )
            recip = rpool.tile([P, 1], F32, tag="recip")
            nc.vector.reciprocal(recip, pv[:, D:D + 1])
            o_sb = opool.tile([P, D], F16, tag="o")
            nc.vector.tensor_scalar_mul(o_sb, in0=pv[:, 0:D], scalar1=recip)
            nc.gpsimd.dma_start(out=out_d[rec["oqs"] + j, :, :], in_=o_sb)
            rec["j"] += 1
            if rec["j"] == rec["nqs"]:
                pending.pop(0)

        col = 0
        oqs = 0
        last_pv = None
        for u, w in enumerate(widths):
            nqs = w // P
            last = (u == nu - 1) and nqs == 1
            e_sb = epool.tile([P, NKT, w], F16, tag="e", name=f"e{u}")
            kt0 = 0
            nch = len(_chunks_for(w))
            for ci, chn in enumerate(_chunks_for(w)):
                ps = ps_s.tile([P, chn, w], F32, tag="ps", name=f"ps{u}_{ci}")
                for j in range(chn):
                    kt = kt0 + j
                    nc.tensor.matmul(
                        ps[:, j, :],
                        lhsT=kt_sb[:, u, kt * P:(kt + 1) * P],
                        rhs=qt_sb[:, col:col + w],
                    )
                nc.scalar.activation(
                    out=e_sb[:, kt0:kt0 + chn, :],
                    in_=ps,
                    func=mybir.ActivationFunctionType.Exp,
                    scale=SCALE,
                )
                kt0 += chn
                emit_pv_step()
                if last:
                    # drain the last unit's PV chunk-by-chunk as each act
                    # lands so the tail after the final act is short
                    if last_pv is None:
                        last_pv = ps_pv.tile([P, D + 1], F32, tag="pv")
                    for kt in range(kt0 - chn, kt0):
                        nc.tensor.matmul(
                            last_pv,
                            lhsT=e_sb[:, kt, 0:P],
                            rhs=vb_sb[:, u, kt, :],
                            start=(kt == 0),
                            stop=(kt == NKT - 1),
                        )
            if last:
                recip = rpool.tile([P, 1], F32, tag="recip")
                nc.vector.reciprocal(recip, last_pv[:, D:D + 1])
                o_sb = opool.tile([P, D], F16, tag="o")
                nc.vector.tensor_scalar_mul(
                    o_sb, in0=last_pv[:, 0:D], scalar1=recip)
                nc.sync.dma_start(out=out_d[oqs, :, :], in_=o_sb)
            else:
                pending.append(
                    {"e": e_sb, "u": u, "nqs": nqs, "oqs": oqs, "j": 0})
            oqs += nqs
            col += w
        while pending:
            emit_pv_step()
    nc.compile()
    return nc


_NC_CACHE = {}


def _get_nc(na, nbm):
    key = (na, nbm)
    if key not in _NC_CACHE:
        _NC_CACHE[key] = _build_program(*key)
    return _NC_CACHE[key]


# ----------------------------------------------------------------------------
# top-level kernel
# ----------------------------------------------------------------------------

def _run(inputs: dict, trace: bool = False):
    q32 = np.ascontiguousarray(np.asarray(inputs["queries"], dtype=np.float32))
    k32 = np.ascontiguousarray(np.asarray(inputs["keys"], dtype=np.float32))
    v32 = np.ascontiguousarray(np.asarray(inputs["values"], dtype=np.float32))
    vl = np.asarray(inputs["valid_lens"]).astype(np.int64)

    if int(vl.sum()) == 0:           # every row masked: output is mean(V)
        meanv = v32.mean(axis=1)
        return np.broadcast_to(meanv[:, None, :], (B, Q, D)).copy(), None

    na, nbm, cores = _plan(vl)
    nc = _get_nc(na, nbm)

    keysT16 = np.ascontiguousarray(
        k32.transpose(0, 2, 1).astype(np.float16))          # [B,128,K]
    vb16 = np.ones((B, P, NKT, D + 1), dtype=np.float16)
    vb16[:, :, :, :D] = (
        v32.reshape(B, NKT, P, D).transpose(0, 2, 1, 3).astype(np.float16))

    in_maps = [
        _core_arrays(units, keysT16, vb16, q32, vl) for units in cores
    ]
    res = run_bass_kernel_spmd(
        nc, in_maps, core_ids=list(range(NCORES)), trace=trace)

    meanv = v32.mean(axis=1)                                 # [B, D]
    out = np.broadcast_to(meanv[:, None, :], (B, Q, D)).copy()
    for c, units in enumerate(cores):
        dev = res.results[c]["out"].astype(np.float32)       # [totqs,128,128]
        qs = 0
        for b, r0, w in units:
            for j in range(w // P):
                if b is not None:
                    lo = r0 + j * P
                    hi = min(int(vl[b]), lo + P)
                    if hi > lo:
                        out[b, lo:hi, :] = dev[qs, 0:hi - lo, :]
                qs += 1
    return out, res


def kernel(**inputs) -> np.ndarray:
    out, _ = _run(inputs, trace=False)
    return out
